# revision 40
# baseline (speedup 1.0000x reference)
"""Distributed multi-head attention kernel for one TRN2 chip (8 NeuronCores).

Problem: x[2,2048,1024] -> qkv proj (W_qkv[3072,1024], b_qkv) -> 16-head
attention (d_key=64) -> out proj (W_o[1024,1024], b_o).

Sharding: head tensor-parallel, 2 heads per core.  Everything on-device is
computed in transposed orientation so no transposes are ever needed:

  per core (heads h0=2c, h1=2c+1):
    qT/kT  [128, 4096]  (2 heads stacked on partitions; cols = b*2048+t),
        computed as  (64*W_q_local) @ x^T  in split-fp8 (see below); the
        64^2 logit scale is folded into the softmax exp's scale operand.
    v      [128pos x 32tile x 130]  natural orientation, with zero-weight /
        bias-64 "ones" columns at 64 and 129 (the 1/64 copy scale turns them
        into 1.0) so the ctx matmul's lhsT = [v_h | ones] produces the
        softmax denominator for free on psum partition 64.
    per (batch, q-quarter) "superiteration", both heads together:
        logitsT[keys,q] = kT-tile-as-lhsT @ qT  (K=64; head0 on PE rows
        0-63, head1 on rows 64-127)
        E = exp(logits/(8*64^2))   (no max subtraction: |l/8| < ~2.5)
        ctxT_unnorm[64,q] + colsum[1,q] accumulated over key tiles in PSUM;
        the ctx matmul for key tile kt is emitted one tile behind its exp
        (two behind across superiteration boundaries) so the in-order PE
        stream never waits on the Activation engine.
        normalize by PE outer-product broadcast of the reciprocal colsum;
        the psum drain / reciprocal / broadcast+mul run as three separate
        fillers inside the NEXT superiteration.

  Projection matmuls run as fp8e4m3 DoubleRow pairs (two 128-feature
  k-tiles per matmul at 0.5 cycles/row).  To stay within fp8's normal
  range the weights are pre-scaled by 64 on the host and split into
  hi + lo fp8 operands (x likewise split unscaled); three cross terms
  (hi*hi, hi*lo, lo*hi) recover ~bf16 accuracy at ~1/3 the PE cost.
  Exact-math bias folds: b_k is dropped entirely (softmax shift
  invariance), b_v is folded into b_o' = b_o + W_o @ b_v on the host,
  b_q is added via a [1,128]-lhsT bias matmul (scaled by 64).

  Output rows are owned interleaved: core c owns rows {m*1024 + c*128 + i,
  m=0..3}, so each row group m's AllToAll fires right after its two
  attention superiterations; the first three collectives and six of the
  eight output-projection groups hide under later attention (each f_op
  trails its collective by two superiterations -- the collective takes
  ~1.4 superiterations and an earlier in-order placement would stall the
  PE on the lw load).  Throwaway fp32 "warm" matmuls keep the PE's
  p-state at maximum through the final collective's ~21us window, and
  the queue assignment keeps data-gated DMAs off the scalar/vector
  queues (their sequencers are shared with the Activation/DVE engines).
  The host scatters the 8x[512,1024] outputs back to natural row order.

Matmul/compute dtype bf16/fp8-split (f32 PSUM accumulation); rel-err gate
is 2e-2.
"""

import sys

sys.path.insert(0, "/opt/trn_rl_repo")

import numpy as np
import ml_dtypes

import concourse.bass as bass
import concourse.tile as tile
from concourse import bacc, mybir
from concourse.bass_utils import run_bass_kernel_spmd

BF16 = mybir.dt.bfloat16
F32 = mybir.dt.float32
FP8 = mybir.dt.float8e4
NPBF16 = ml_dtypes.bfloat16
NPFP8 = ml_dtypes.float8_e4m3
DR = mybir.MatmulPerfMode.DoubleRow

D = 1024  # d_model
T = 2048  # seq len
B = 2  # batch
P = B * T  # 4096 total positions
H = 16  # total heads
DK = 64  # head dim
NCORES = 8
HL = H // NCORES  # 2 heads per core

WSCALE = 64.0  # host-side weight scale keeping fp8 operands in normal range
EXP_SCALE = 0.125 / (WSCALE * WSCALE)  # softmax exp scale incl. 64^2 logits


def build_graph(reps=1):
    nc = bacc.Bacc(
        "TRN2", target_bir_lowering=False, debug=False, num_devices=NCORES
    )

    # --- per-core external inputs ---
    x_hi = nc.declare_dram_parameter("x_hi", [D, P], FP8, isOutput=False)
    x_lo = nc.declare_dram_parameter("x_lo", [D, P], FP8, isOutput=False)
    wq_hi = nc.declare_dram_parameter("wq_hi", [D, 128], FP8, isOutput=False)
    wq_lo = nc.declare_dram_parameter("wq_lo", [D, 128], FP8, isOutput=False)
    wk_hi = nc.declare_dram_parameter("wk_hi", [D, 128], FP8, isOutput=False)
    wk_lo = nc.declare_dram_parameter("wk_lo", [D, 128], FP8, isOutput=False)
    wv_hi = nc.declare_dram_parameter("wv_hi", [D, 130], FP8, isOutput=False)
    wv_lo = nc.declare_dram_parameter("wv_lo", [D, 130], FP8, isOutput=False)
    bq = nc.declare_dram_parameter("bq", [1, 128], BF16, isOutput=False)
    bv = nc.declare_dram_parameter("bv", [1, 130], BF16, isOutput=False)
    woT = nc.declare_dram_parameter("woT", [D, D], BF16, isOutput=False)
    bo = nc.declare_dram_parameter("bo", [1, D], BF16, isOutput=False)
    out = nc.declare_dram_parameter("out", [P // NCORES, D], F32, isOutput=True)

    with tile.TileContext(nc) as tc:
        with (
            tc.tile_pool(name="const", bufs=1) as const_pool,
            tc.tile_pool(name="xw", bufs=1) as xw_pool,
            tc.tile_pool(name="qkv", bufs=1) as qkv_pool,
            tc.tile_pool(name="et", bufs=3) as et_pool,
            tc.tile_pool(name="norm", bufs=2) as norm_pool,
            tc.tile_pool(name="ctxn", bufs=4) as ctxn_pool,
            tc.tile_pool(name="ow", bufs=2) as ow_pool,
            tc.tile_pool(name="obuf", bufs=2) as obuf_pool,
            tc.tile_pool(name="ps_mm", bufs=2, space="PSUM") as ps_mm,
            tc.tile_pool(name="ps_log", bufs=2, space="PSUM") as ps_log,
            tc.tile_pool(name="ps_ctx", bufs=2, space="PSUM") as ps_ctx,
            tc.tile_pool(name="dram", bufs=1, space="DRAM") as dram_pool,
        ):
            # tiny constants on the gpsimd queue so sync/scalar HWDGE start
            # on the weights/x immediately
            bq_sb = const_pool.tile([1, 128], BF16)
            bv_sb = const_pool.tile([1, 130], BF16)
            bo_sb = const_pool.tile([1, D], BF16)
            nc.gpsimd.dma_start(out=bv_sb, in_=bv[:, :])
            nc.gpsimd.dma_start(out=bq_sb, in_=bq[:, :])
            ones_sb = const_pool.tile([1, 128], BF16)
            nc.vector.memset(ones_sb, 1.0)
            ones512_sb = const_pool.tile([1, 512], BF16)
            nc.vector.memset(ones512_sb, 1.0)
            ones65_sb = const_pool.tile([65, 128], BF16)
            nc.vector.memset(ones65_sb, 1.0)

            env = dict(locals())
            env.pop("env", None)
            for rep in range(reps):
                emit_body(nc, tc, env)

    nc.compile()
    return nc


def emit_body(nc, tc, env):
    """Emit one full forward pass.

    Engine instruction streams are in-order, so the emission schedule is a
    hand software-pipeline.  Attention processes BOTH local heads together
    per 512-column q-quarter; QKV projection groups and output-projection
    groups are interleaved as fillers into the attention kt-loops.
    """
    g = type("G", (), env)  # attribute access to captured bindings

    x_hi, x_lo = g.x_hi, g.x_lo
    wq_hi, wq_lo, wk_hi, wk_lo = g.wq_hi, g.wq_lo, g.wk_hi, g.wk_lo
    wv_hi, wv_lo, woT, out = g.wv_hi, g.wv_lo, g.woT, g.out
    bq_sb, bv_sb, bo_sb = g.bq_sb, g.bv_sb, g.bo_sb
    ones_sb, ones512_sb, ones65_sb = g.ones_sb, g.ones512_sb, g.ones65_sb
    xw_pool, qkv_pool = g.xw_pool, g.qkv_pool
    et_pool, norm_pool, ctxn_pool = g.et_pool, g.norm_pool, g.ctxn_pool
    ow_pool, obuf_pool = g.ow_pool, g.obuf_pool
    ps_mm, ps_log, ps_ctx, dram_pool = g.ps_mm, g.ps_log, g.ps_ctx, g.dram_pool

    # --- weights into SBUF first (k/q weights gate the first projections) ---
    wkh_sb = xw_pool.tile([128, 8, 128], FP8, name="wkh_sb")
    wkl_sb = xw_pool.tile([128, 8, 128], FP8, name="wkl_sb")
    wqh_sb = xw_pool.tile([128, 8, 128], FP8, name="wqh_sb")
    wql_sb = xw_pool.tile([128, 8, 128], FP8, name="wql_sb")
    wvh_sb = xw_pool.tile([128, 8, 130], FP8, name="wvh_sb")
    wvl_sb = xw_pool.tile([128, 8, 130], FP8, name="wvl_sb")
    nc.sync.dma_start(out=wkh_sb, in_=wk_hi[:, :].rearrange("(a p) c -> p a c", p=128))
    nc.scalar.dma_start(out=wqh_sb, in_=wq_hi[:, :].rearrange("(a p) c -> p a c", p=128))

    # --- x^T hi/lo; chunk 0 lands as kt-pair pieces so the first DoubleRow
    #     matmul can start as soon as its pair arrives; hi rides the sync
    #     queue, lo the scalar queue ---
    xh_sb = xw_pool.tile([128, 8, P], FP8, name="xh_sb")  # [part, ktile, pos]
    xl_sb = xw_pool.tile([128, 8, P], FP8, name="xl_sb")
    xh_r = x_hi[:, :].rearrange("(a p) c -> p a c", p=128)
    xl_r = x_lo[:, :].rearrange("(a p) c -> p a c", p=128)
    for tp in range(2):
        kp = slice(4 * tp, 4 * tp + 4)
        nc.sync.dma_start(out=xh_sb[:, kp, 0:512], in_=xh_r[:, kp, 0:512])
        nc.scalar.dma_start(out=xl_sb[:, kp, 0:512], in_=xl_r[:, kp, 0:512])
    nc.sync.dma_start(out=wkl_sb, in_=wk_lo[:, :].rearrange("(a p) c -> p a c", p=128))
    nc.scalar.dma_start(out=wql_sb, in_=wq_lo[:, :].rearrange("(a p) c -> p a c", p=128))
    nc.sync.dma_start(
        out=wvh_sb, in_=wv_hi[:, :].rearrange("(a p) c -> p a c", p=128)
    )
    nc.scalar.dma_start(
        out=wvl_sb, in_=wv_lo[:, :].rearrange("(a p) c -> p a c", p=128)
    )
    nc.gpsimd.dma_start(out=g.bo_sb, in_=g.bo[:, :])
    # late x-lo chunks ride sync: the scalar queue shares the Activation
    # sequencer, and dispatches there delay the first superiteration's exps
    for cb in range(1, 8):
        csl = slice(cb * 512, (cb + 1) * 512)
        nc.sync.dma_start(out=xh_sb[:, :, csl], in_=xh_r[:, :, csl])
        eng = nc.scalar if cb <= 2 else nc.sync
        eng.dma_start(out=xl_sb[:, :, csl], in_=xl_r[:, :, csl])

    # W_o isn't needed until the first output projection (~half-way in)
    wo_sb = ow_pool.tile([128, 8, D], BF16, name="wo_sb")
    woT_r = woT[:, :].rearrange("(a p) c -> p a c", p=128)
    nc.sync.dma_start(out=wo_sb[:, 0:4, :], in_=woT_r[:, 0:4, :])
    nc.scalar.dma_start(out=wo_sb[:, 4:8, :], in_=woT_r[:, 4:8, :])

    q_sb = qkv_pool.tile([128, P], BF16, name="q_sb")
    k_sb = qkv_pool.tile([128, P], BF16, name="k_sb")
    v_sb = qkv_pool.tile([128, 32, 130], BF16, name="v_sb")

    # Per-row-group A2A buffers: group m = rows m*1024 + c*128 .. +128.
    cc_in = [
        dram_pool.tile([NCORES * 128, 128], BF16, name=f"cc_in{m}") for m in range(4)
    ]
    tmp = [
        dram_pool.tile([NCORES * 128, 128], BF16, name=f"tmp{m}") for m in range(4)
    ]
    lw_all = [None] * 4

    # ---- filler units: one PSUM group each, emitted inside attention ----
    def dr3(ps, wh, wl, xh, xl, sl, last_stop):
        """12 DoubleRow matmuls: hi*hi + hi*lo + lo*hi over 4 kt-pairs."""
        terms = [(wh, xh), (wh, xl), (wl, xh)]
        for ti, (w, x) in enumerate(terms):
            for tp in range(4):
                kp = slice(2 * tp, 2 * tp + 2)
                nc.tensor.matmul(
                    out=ps,
                    lhsT=w[:, kp, :],
                    rhs=x[:, kp, sl],
                    start=(ti == 0 and tp == 0),
                    stop=(last_stop and ti == 2 and tp == 3),
                    perf_mode=DR,
                )

    def f_k(p8):
        def emit():
            sl = slice(p8 * 512, (p8 + 1) * 512)
            ps = ps_mm.tile([128, 512], F32, tag="mm", name="ps_k")
            dr3(ps, wkh_sb, wkl_sb, xh_sb, xl_sb, sl, last_stop=True)
            nc.vector.tensor_copy(out=k_sb[:, sl], in_=ps)
        return emit

    def f_q(p8):
        def emit():
            sl = slice(p8 * 512, (p8 + 1) * 512)
            ps = ps_mm.tile([128, 512], F32, tag="mm", name="ps_q")
            dr3(ps, wqh_sb, wql_sb, xh_sb, xl_sb, sl, last_stop=False)
            nc.tensor.matmul(
                out=ps, lhsT=bq_sb, rhs=ones512_sb, start=False, stop=True
            )
            nc.vector.tensor_copy(out=q_sb[:, sl], in_=ps)
        return emit

    def f_v(pt):
        def emit():
            psl = slice(pt * 128, (pt + 1) * 128)
            ps = ps_mm.tile([128, 130], F32, tag="mm", name="ps_v")
            terms = [(xh_sb, wvh_sb), (xh_sb, wvl_sb), (xl_sb, wvh_sb)]
            for ti, (x, w) in enumerate(terms):
                for tp in range(4):
                    kp = slice(2 * tp, 2 * tp + 2)
                    nc.tensor.matmul(
                        out=ps,
                        lhsT=x[:, kp, psl],
                        rhs=w[:, kp, :],
                        start=(ti == 0 and tp == 0),
                        stop=False,
                        perf_mode=DR,
                    )
            nc.tensor.matmul(
                out=ps, lhsT=ones_sb[:, 0:128], rhs=bv_sb, start=False, stop=True
            )
            with nc.allow_low_precision(reason="v copy with 1/64 descale"):
                nc.vector.tensor_scalar_mul(
                    out=v_sb[:, pt, :], in0=ps, scalar1=1.0 / WSCALE
                )
        return emit

    def f_lw(m, fast=False):
        def emit():
            t = obuf_pool.tile([128, 8, 128], BF16, tag=f"lw{m}", name="lw")
            src = tmp[m][:, :].rearrange("(a p) r -> p a r", p=128)
            if fast:
                # tail-critical load: quarters alternating between the two
                # HWDGE queues so the first kt-blocks land early
                for qt, eng in enumerate([nc.sync, nc.scalar, nc.sync,
                                          nc.scalar]):
                    eng.dma_start(out=t[:, 2 * qt : 2 * qt + 2, :],
                                  in_=src[:, 2 * qt : 2 * qt + 2, :])
            else:
                nc.gpsimd.dma_start(out=t[:, 0:4, :], in_=src[:, 0:4, :])
                nc.gpsimd.dma_start(out=t[:, 4:8, :], in_=src[:, 4:8, :])
            lw_all[m] = t
        return emit

    def f_op(m, nt, split_out=False):
        def emit():
            lw = lw_all[m]
            ps = ps_mm.tile([128, 512], F32, tag="mm", name="ps_o")
            for kt in range(8):
                nc.tensor.matmul(
                    out=ps, lhsT=lw[:, kt, :],
                    rhs=wo_sb[:, kt, nt * 512 : (nt + 1) * 512],
                    start=(kt == 0), stop=False,
                )
            nc.tensor.matmul(
                out=ps, lhsT=ones_sb, rhs=bo_sb[:, nt * 512 : (nt + 1) * 512],
                start=False, stop=True,
            )
            o_sb = obuf_pool.tile([128, 512], F32, tag="ob", name="o_sb")
            if split_out:
                for h, eng in enumerate([nc.sync, nc.scalar]):
                    sl = slice(h * 256, (h + 1) * 256)
                    nc.vector.tensor_copy(out=o_sb[:, sl], in_=ps[:, sl])
                    eng.dma_start(
                        out=out[m * 128 : (m + 1) * 128,
                                nt * 512 + h * 256 : nt * 512 + (h + 1) * 256],
                        in_=o_sb[:, sl],
                    )
            else:
                nc.vector.tensor_copy(out=o_sb, in_=ps)
                nc.sync.dma_start(
                    out=out[m * 128 : (m + 1) * 128,
                            nt * 512 : (nt + 1) * 512],
                    in_=o_sb,
                )
        return emit

    def emit_a2a(m):
        nc.gpsimd.collective_compute(
            "AllToAll",
            mybir.AluOpType.bypass,
            replica_groups=[list(range(NCORES))],
            ins=[cc_in[m][:].opt()],
            outs=[tmp[m][:].opt()],
        )

    # pending ctx matmul state: the ctx accumulation for key tile kt is
    # emitted one kt later (after the NEXT tile's logits) -- two tiles later
    # across a superiteration boundary -- so the PE never stalls on the
    # exp's completion semaphore or the previous si's PSUM drain.
    pend = []

    def flush_ctx():
        for b, kt, ps_c, et in pend:
            for hh in range(2):
                nc.tensor.matmul(
                    out=ps_c[hh],
                    lhsT=v_sb[:, b * 16 + kt, 65 * hh : 65 * hh + 65],
                    rhs=et[:, hh * 512 : (hh + 1) * 512],
                    start=(kt == 0),
                    stop=(kt == 15),
                )
        pend.clear()

    def emit_attn_part(b, qq, ps_c, kts, fillers=()):
        """Key tiles kts of one superiteration (both heads, q cols qq*512..)."""
        fillers = list(fillers)
        co = b * T
        qco = co + qq * 512
        nf = 0
        nkt = len(kts)
        for ki, kt in enumerate(kts):
            ps_l = ps_log.tile([128, 1024], F32, tag="log", name="ps_l")
            for hh in range(2):
                po = DK * hh
                nc.tensor.matmul(
                    out=ps_l[:, hh * 512 : (hh + 1) * 512],
                    lhsT=k_sb[po : po + DK, co + kt * 128 : co + (kt + 1) * 128],
                    rhs=q_sb[po : po + DK, qco : qco + 512],
                    start=True,
                    stop=True,
                )
            if kt != 1:  # lag-2 across the superiteration boundary
                flush_ctx()
            want = (ki + 1) * len(fillers) // nkt
            while nf < want:
                fillers[nf]()
                nf += 1
            et = et_pool.tile([128, 1024], BF16, tag="et", name="et")
            nc.scalar.activation(
                out=et, in_=ps_l,
                func=mybir.ActivationFunctionType.Exp,
                scale=EXP_SCALE,
            )
            pend.append((b, kt, ps_c, et))

    def emit_norm_copies(b, qq, ps_c):
        """DVE-only PSUM drain at the end of a superiteration (releases the
        ctx psum banks on the baseline schedule)."""
        flush_ctx()
        ctxr = norm_pool.tile([65, 1024], F32, tag="ctxr", name="ctxr")
        for hh in range(2):
            nc.vector.tensor_copy(
                out=ctxr[:, hh * 512 : (hh + 1) * 512], in_=ps_c[hh]
            )
        return ctxr

    def f_norm(b, qq, ctxr):
        """Deferred normalize, split in two fillers: [0] reciprocal (DVE
        only), [1] PE broadcast + mul + scatter.  Placing them a few key
        tiles apart in the next superiteration keeps the PE stream from
        ever waiting on the reciprocal."""
        m = 2 * b + qq // 2
        half = qq % 2
        rs = norm_pool.tile([65, 1024], BF16, tag="rsum", name="rs")

        def emit_recip():
            with nc.allow_low_precision(reason="softmax denom bf16 bcast"):
                nc.vector.reciprocal(out=rs[64:65, :], in_=ctxr[64:65, :])

        def emit_mul():
            ctxn = ctxn_pool.tile([64, 1024], BF16, tag="cn", name="ctxn")
            for hh in range(2):
                bc = ps_mm.tile([64, 512], F32, tag="mm", name="bc")
                nc.tensor.matmul(
                    out=bc,
                    lhsT=ones65_sb[64:65, 0:64],
                    rhs=rs[64:65, hh * 512 : (hh + 1) * 512],
                    start=True,
                    stop=True,
                )
                nc.vector.tensor_mul(
                    out=ctxn[:, hh * 512 : (hh + 1) * 512],
                    in0=ctxr[0:64, hh * 512 : (hh + 1) * 512],
                    in1=bc,
                )
                nc.sync.dma_start(
                    out=cc_in[m][:, :].rearrange("(j q) r -> q j r", q=128)[
                        DK * hh : DK * hh + DK, half * 4 : half * 4 + 4, :
                    ],
                    in_=ctxn[:, hh * 512 : (hh + 1) * 512].rearrange(
                        "f (j r) -> f j r", j=4
                    ),
                )
        return emit_recip, emit_mul

    def emit_norm_inline(b, qq, ps_c, prewarm=2):
        """Latency-critical norm (the very last superiteration): reciprocals
        read the colsum rows straight from PSUM, with PE keep-alive matmuls
        covering their latency."""
        flush_ctx()
        m = 2 * b + qq // 2
        half = qq % 2
        warm(prewarm)
        rs = norm_pool.tile([65, 1024], BF16, tag="rsum", name="rs")
        with nc.allow_low_precision(reason="softmax denom bf16 broadcast"):
            for hh in range(2):
                nc.vector.reciprocal(
                    out=rs[64:65, hh * 512 : (hh + 1) * 512],
                    in_=ps_c[hh][64:65, :],
                )
        ctxr = norm_pool.tile([65, 1024], F32, tag="ctxr", name="ctxr")
        for hh in range(2):
            nc.vector.tensor_copy(
                out=ctxr[:, hh * 512 : (hh + 1) * 512], in_=ps_c[hh]
            )
        ctxn = ctxn_pool.tile([64, 1024], BF16, tag="cn", name="ctxn")
        for hh in range(2):
            bc = ps_mm.tile([64, 512], F32, tag="mm", name="bc")
            nc.tensor.matmul(
                out=bc,
                lhsT=ones65_sb[64:65, 0:64],
                rhs=rs[64:65, hh * 512 : (hh + 1) * 512],
                start=True,
                stop=True,
            )
            nc.vector.tensor_mul(
                out=ctxn[:, hh * 512 : (hh + 1) * 512],
                in0=ctxr[0:64, hh * 512 : (hh + 1) * 512],
                in1=bc,
            )
            nc.sync.dma_start(
                out=cc_in[m][:, :].rearrange("(j q) r -> q j r", q=128)[
                    DK * hh : DK * hh + DK, half * 4 : half * 4 + 4, :
                ],
                in_=ctxn[:, hh * 512 : (hh + 1) * 512].rearrange(
                    "f (j r) -> f j r", j=4
                ),
            )

    def alloc_ps_c():
        return [
            ps_ctx.tile([65, 512], F32, tag="ctx", name=f"psc{hh}")
            for hh in range(2)
        ]

    def emit_attn(b, qq, fillers=()):
        """One full superiteration: both heads, q columns qq*512..+512.
        Returns three deferred fillers (flush+psum-drain, recip, broadcast+
        mul+scatter) for the next superiteration, so the si boundary never
        stalls the PE on the exp/normalize chain."""
        ps_c = alloc_ps_c()
        emit_attn_part(b, qq, ps_c, range(16), fillers)
        ctxr = norm_pool.tile([65, 1024], F32, tag="ctxr", name="ctxr")

        def copies():
            flush_ctx()
            for hh in range(2):
                nc.vector.tensor_copy(
                    out=ctxr[:, hh * 512 : (hh + 1) * 512], in_=ps_c[hh]
                )
        r, mu = f_norm(b, qq, ctxr)
        return copies, r, mu

    # fp32 warm source for PE keep-alive matmuls (throwaway work used to
    # cover reciprocal latencies and the final collective window -- a cold
    # PE restarts at half/quarter p-state)
    warm_src = obuf_pool.tile([128, 512], F32, tag="warm", name="warm_src")

    def warm(n):
        for _ in range(n):
            ps_d = ps_log.tile([128, 512], F32, tag="log", name="ps_warm")
            nc.tensor.matmul(
                out=ps_d, lhsT=warm_src[:, 0:128], rhs=warm_src,
                start=True, stop=True,
            )

        noop = lambda: None

    # ---- emission schedule ----
    # superiteration (0,0) is streamed in 4-kt blocks: each block's k slice
    # and v tiles are emitted (top level) just before the part that consumes
    # them, so attention starts as soon as x chunk 0 lands.  Output
    # projections trail their group's AllToAll by two superiterations (the
    # collective takes ~1.4 superiterations; anything earlier stalls the
    # in-order PE queue on the lw load).
    f_k(0)()
    f_q(0)()
    for pt in range(4):
        f_v(pt)()
    nc.vector.tensor_copy(out=warm_src, in_=q_sb[:, 0:512])
    ps_c00 = alloc_ps_c()
    for blk in range(4):
        if blk < 3:
            f_k(blk + 1)()
            for pt in range(4 * blk + 4, 4 * blk + 8):
                f_v(pt)()
        emit_attn_part(0, 0, ps_c00, range(4 * blk, 4 * blk + 4),
                       [f_q(1)] if blk == 3 else [])
    ctxr00 = emit_norm_copies(0, 0, ps_c00)
    r00, m00 = f_norm(0, 0, ctxr00)
    c01, r01, m01 = emit_attn(0, 1, [r00, m00, f_q(2), f_k(4), f_k(5),
                                     f_k(6), f_k(7)])
    c02, r02, m02 = emit_attn(0, 2, [c01, r01, m01, lambda: emit_a2a(0),
                                     f_q(3)]
                              + [f_v(pt) for pt in range(16, 24)])
    c03, r03, m03 = emit_attn(0, 3, [c02, r02, m02]
                              + [f_v(pt) for pt in range(24, 32)]
                              + [f_q(4), f_lw(0)])
    c10, r10, m10 = emit_attn(1, 0, [c03, r03, m03, f_q(5),
                                     f_op(0, 0), lambda: emit_a2a(1)])
    c11, r11, m11 = emit_attn(1, 1, [c10, r10, m10, f_q(6), f_op(0, 1),
                                     f_lw(1)])
    c12, r12, m12 = emit_attn(1, 2, [c11, r11, m11, f_q(7),
                                     f_op(1, 0), lambda: emit_a2a(2)])
    ps_c13 = alloc_ps_c()
    emit_attn_part(1, 3, ps_c13, range(16), [c12, r12, m12, f_op(1, 1), f_lw(2)])
    emit_norm_inline(1, 3, ps_c13, prewarm=2)
    emit_a2a(3)
    warm(3)  # bridges until lw2 lands (gated by the m2 collective)
    f_op(2, 0)()
    f_op(2, 1)()
    # keep the PE hot through the last collective's ~21us window
    warm(30)
    f_lw(3, fast=True)()
    f_op(3, 0)()
    f_op(3, 1, split_out=True)()


def _fp8_split(a):
    """a (f32) -> (hi, lo) fp8e4m3 with hi + lo ~= a."""
    hi = a.astype(NPFP8)
    lo = (a - hi.astype(np.float32)).astype(NPFP8)
    return hi, lo


def make_in_maps(x, W_qkv, b_qkv, W_o, b_o):
    x = np.asarray(x, dtype=np.float32)
    W_qkv = np.asarray(W_qkv, dtype=np.float32)
    b_qkv = np.asarray(b_qkv, dtype=np.float32)
    W_o = np.asarray(W_o, dtype=np.float32)
    b_o = np.asarray(b_o, dtype=np.float32)

    xT = np.ascontiguousarray(x.reshape(P, D).T)
    x_hi, x_lo = _fp8_split(xT)
    woT = np.ascontiguousarray(W_o.T).astype(NPBF16)
    # fold b_v into the output bias: out += W_o @ b_v
    bv_full = b_qkv[2 * D : 3 * D]
    bo_eff = (b_o + W_o @ bv_full).reshape(1, D).astype(NPBF16)

    in_maps = []
    for c in range(NCORES):
        wq = W_qkv[128 * c : 128 * c + 128]  # [128, 1024] q features
        wk = W_qkv[D + 128 * c : D + 128 * c + 128]
        wv = W_qkv[2 * D + 128 * c : 2 * D + 128 * c + 128]
        wvT_pad = np.zeros((D, 130), dtype=np.float32)
        wvT_pad[:, 0:64] = wv[0:64].T
        wvT_pad[:, 65:129] = wv[64:128].T
        # ones columns come from the bias matmul (value 64 -> 1.0 after the
        # 1/64 copy descale); v's real bias was folded into bo_eff.
        bv_pad = np.zeros((1, 130), dtype=np.float32)
        bv_pad[0, 64] = WSCALE
        bv_pad[0, 129] = WSCALE
        wq_hi, wq_lo = _fp8_split(WSCALE * np.ascontiguousarray(wq.T))
        wk_hi, wk_lo = _fp8_split(WSCALE * np.ascontiguousarray(wk.T))
        wv_hi, wv_lo = _fp8_split(WSCALE * wvT_pad)
        in_maps.append(
            {
                "x_hi": x_hi,
                "x_lo": x_lo,
                "wq_hi": wq_hi,
                "wq_lo": wq_lo,
                "wk_hi": wk_hi,
                "wk_lo": wk_lo,
                "wv_hi": wv_hi,
                "wv_lo": wv_lo,
                "bq": (WSCALE * b_qkv[128 * c : 128 * c + 128])
                .reshape(1, 128)
                .astype(NPBF16),
                "bv": bv_pad.astype(NPBF16),
                "woT": woT,
                "bo": bo_eff,
            }
        )
    return in_maps


def assemble_out(outs):
    """outs[c] is [512, 1024]: row tile rt holds global rows
    rt*1024 + c*128 .. +128 (interleaved ownership)."""
    full = np.zeros((P, D), dtype=np.float32)
    for c in range(NCORES):
        oc = np.asarray(outs[c], dtype=np.float32)
        for rt in range(4):
            full[rt * 1024 + c * 128 : rt * 1024 + c * 128 + 128] = oc[
                rt * 128 : (rt + 1) * 128
            ]
    return full.reshape(B, T, D)


_CACHED_GRAPH = None


def kernel(x, W_qkv, b_qkv, W_o, b_o):
    global _CACHED_GRAPH
    if _CACHED_GRAPH is None:
        _CACHED_GRAPH = build_graph()
    nc = _CACHED_GRAPH
    in_maps = make_in_maps(x, W_qkv, b_qkv, W_o, b_o)
    res = run_bass_kernel_spmd(nc, in_maps, core_ids=list(range(NCORES)))
    outs = [res.results[c]["out"] for c in range(NCORES)]
    return assemble_out(outs)


# revision 43
# speedup vs baseline: 1.0041x; 1.0041x over previous
"""Distributed multi-head attention kernel for one TRN2 chip (8 NeuronCores).

Problem: x[2,2048,1024] -> qkv proj (W_qkv[3072,1024], b_qkv) -> 16-head
attention (d_key=64) -> out proj (W_o[1024,1024], b_o).

Sharding: head tensor-parallel, 2 heads per core.  Everything on-device is
computed in transposed orientation so no transposes are ever needed:

  per core (heads h0=2c, h1=2c+1):
    qT/kT  [128, 4096]  (2 heads stacked on partitions; cols = b*2048+t),
        computed as  (64*W_q_local) @ x^T  in split-fp8 (see below); the
        64^2 logit scale is folded into the softmax exp's scale operand.
    v      [128pos x 32tile x 130]  natural orientation, with zero-weight /
        bias-64 "ones" columns at 64 and 129 (the 1/64 copy scale turns them
        into 1.0) so the ctx matmul's lhsT = [v_h | ones] produces the
        softmax denominator for free on psum partition 64.
    per (batch, q-quarter) "superiteration", both heads together:
        logitsT[keys,q] = kT-tile-as-lhsT @ qT  (K=64; head0 on PE rows
        0-63, head1 on rows 64-127)
        E = exp(logits/(8*64^2))   (no max subtraction: |l/8| < ~2.5)
        ctxT_unnorm[64,q] + colsum[1,q] accumulated over key tiles in PSUM;
        the ctx matmul for key tile kt is emitted one tile behind its exp
        (two behind across superiteration boundaries) so the in-order PE
        stream never waits on the Activation engine.
        normalize by PE outer-product broadcast of the reciprocal colsum;
        the psum drain / reciprocal / broadcast+mul run as three separate
        fillers inside the NEXT superiteration.

  Projection matmuls run as fp8e4m3 DoubleRow pairs (two 128-feature
  k-tiles per matmul at 0.5 cycles/row).  To stay within fp8's normal
  range the weights are pre-scaled by 64 on the host and split into
  hi + lo fp8 operands (x likewise split unscaled); three cross terms
  (hi*hi, hi*lo, lo*hi) recover ~bf16 accuracy at ~1/3 the PE cost.
  Exact-math bias folds: b_k is dropped entirely (softmax shift
  invariance), b_v is folded into b_o' = b_o + W_o @ b_v on the host,
  b_q is added via a [1,128]-lhsT bias matmul (scaled by 64).

  Output rows are owned interleaved: core c owns rows {m*1024 + c*128 + i,
  m=0..3}, so each row group m's AllToAll fires right after its two
  attention superiterations; the first three collectives and six of the
  eight output-projection groups hide under later attention (each f_op
  trails its collective by two superiterations -- the collective takes
  ~1.4 superiterations and an earlier in-order placement would stall the
  PE on the lw load).  Throwaway fp32 "warm" matmuls keep the PE's
  p-state at maximum through the final collective's ~21us window, and
  the queue assignment keeps data-gated DMAs off the scalar/vector
  queues (their sequencers are shared with the Activation/DVE engines).
  The host scatters the 8x[512,1024] outputs back to natural row order.

Matmul/compute dtype bf16/fp8-split (f32 PSUM accumulation); rel-err gate
is 2e-2.
"""

import sys

sys.path.insert(0, "/opt/trn_rl_repo")

import numpy as np
import ml_dtypes

import concourse.bass as bass
import concourse.tile as tile
from concourse import bacc, mybir
from concourse.bass_utils import run_bass_kernel_spmd

BF16 = mybir.dt.bfloat16
F32 = mybir.dt.float32
FP8 = mybir.dt.float8e4
NPBF16 = ml_dtypes.bfloat16
NPFP8 = ml_dtypes.float8_e4m3
DR = mybir.MatmulPerfMode.DoubleRow

D = 1024  # d_model
T = 2048  # seq len
B = 2  # batch
P = B * T  # 4096 total positions
H = 16  # total heads
DK = 64  # head dim
NCORES = 8
HL = H // NCORES  # 2 heads per core

WSCALE = 64.0  # host-side weight scale keeping fp8 operands in normal range
EXP_SCALE = 0.125 / (WSCALE * WSCALE)  # softmax exp scale incl. 64^2 logits


def build_graph(reps=1):
    nc = bacc.Bacc(
        "TRN2", target_bir_lowering=False, debug=False, num_devices=NCORES
    )

    # --- per-core external inputs ---
    x_hi = nc.declare_dram_parameter("x_hi", [D, P], FP8, isOutput=False)
    x_lo = nc.declare_dram_parameter("x_lo", [D, P], FP8, isOutput=False)
    wq_hi = nc.declare_dram_parameter("wq_hi", [D, 128], FP8, isOutput=False)
    wq_lo = nc.declare_dram_parameter("wq_lo", [D, 128], FP8, isOutput=False)
    wk_hi = nc.declare_dram_parameter("wk_hi", [D, 128], FP8, isOutput=False)
    wk_lo = nc.declare_dram_parameter("wk_lo", [D, 128], FP8, isOutput=False)
    wv_hi = nc.declare_dram_parameter("wv_hi", [D, 130], FP8, isOutput=False)
    wv_lo = nc.declare_dram_parameter("wv_lo", [D, 130], FP8, isOutput=False)
    bq = nc.declare_dram_parameter("bq", [1, 128], BF16, isOutput=False)
    bv = nc.declare_dram_parameter("bv", [1, 130], BF16, isOutput=False)
    woT = nc.declare_dram_parameter("woT", [D, D], BF16, isOutput=False)
    bo = nc.declare_dram_parameter("bo", [1, D], BF16, isOutput=False)
    out = nc.declare_dram_parameter("out", [P // NCORES, D], F32, isOutput=True)

    with tile.TileContext(nc) as tc:
        with (
            tc.tile_pool(name="const", bufs=1) as const_pool,
            tc.tile_pool(name="xw", bufs=1) as xw_pool,
            tc.tile_pool(name="qkv", bufs=1) as qkv_pool,
            tc.tile_pool(name="et", bufs=3) as et_pool,
            tc.tile_pool(name="norm", bufs=2) as norm_pool,
            tc.tile_pool(name="ctxn", bufs=4) as ctxn_pool,
            tc.tile_pool(name="ow", bufs=2) as ow_pool,
            tc.tile_pool(name="obuf", bufs=2) as obuf_pool,
            tc.tile_pool(name="ps_mm", bufs=2, space="PSUM") as ps_mm,
            tc.tile_pool(name="ps_log", bufs=2, space="PSUM") as ps_log,
            tc.tile_pool(name="ps_ctx", bufs=2, space="PSUM") as ps_ctx,
            tc.tile_pool(name="dram", bufs=1, space="DRAM") as dram_pool,
        ):
            # tiny constants on the gpsimd queue so sync/scalar HWDGE start
            # on the weights/x immediately
            bq_sb = const_pool.tile([1, 128], BF16)
            bv_sb = const_pool.tile([1, 130], BF16)
            bo_sb = const_pool.tile([1, D], BF16)
            nc.gpsimd.dma_start(out=bv_sb, in_=bv[:, :])
            nc.gpsimd.dma_start(out=bq_sb, in_=bq[:, :])
            ones_sb = const_pool.tile([1, 128], BF16)
            nc.vector.memset(ones_sb, 1.0)
            ones512_sb = const_pool.tile([1, 512], BF16)
            nc.vector.memset(ones512_sb, 1.0)
            ones65_sb = const_pool.tile([65, 128], BF16)
            nc.vector.memset(ones65_sb, 1.0)

            env = dict(locals())
            env.pop("env", None)
            for rep in range(reps):
                emit_body(nc, tc, env)

    nc.compile()
    return nc


def emit_body(nc, tc, env):
    """Emit one full forward pass.

    Engine instruction streams are in-order, so the emission schedule is a
    hand software-pipeline.  Attention processes BOTH local heads together
    per 512-column q-quarter; QKV projection groups and output-projection
    groups are interleaved as fillers into the attention kt-loops.
    """
    g = type("G", (), env)  # attribute access to captured bindings

    x_hi, x_lo = g.x_hi, g.x_lo
    wq_hi, wq_lo, wk_hi, wk_lo = g.wq_hi, g.wq_lo, g.wk_hi, g.wk_lo
    wv_hi, wv_lo, woT, out = g.wv_hi, g.wv_lo, g.woT, g.out
    bq_sb, bv_sb, bo_sb = g.bq_sb, g.bv_sb, g.bo_sb
    ones_sb, ones512_sb, ones65_sb = g.ones_sb, g.ones512_sb, g.ones65_sb
    xw_pool, qkv_pool = g.xw_pool, g.qkv_pool
    et_pool, norm_pool, ctxn_pool = g.et_pool, g.norm_pool, g.ctxn_pool
    ow_pool, obuf_pool = g.ow_pool, g.obuf_pool
    ps_mm, ps_log, ps_ctx, dram_pool = g.ps_mm, g.ps_log, g.ps_ctx, g.dram_pool

    # --- weights into SBUF first (k/q weights gate the first projections) ---
    wkh_sb = xw_pool.tile([128, 8, 128], FP8, name="wkh_sb")
    wkl_sb = xw_pool.tile([128, 8, 128], FP8, name="wkl_sb")
    wqh_sb = xw_pool.tile([128, 8, 128], FP8, name="wqh_sb")
    wql_sb = xw_pool.tile([128, 8, 128], FP8, name="wql_sb")
    wvh_sb = xw_pool.tile([128, 8, 130], FP8, name="wvh_sb")
    wvl_sb = xw_pool.tile([128, 8, 130], FP8, name="wvl_sb")
    nc.sync.dma_start(out=wkh_sb, in_=wk_hi[:, :].rearrange("(a p) c -> p a c", p=128))
    nc.scalar.dma_start(out=wqh_sb, in_=wq_hi[:, :].rearrange("(a p) c -> p a c", p=128))

    # --- x^T hi/lo; chunk 0 lands as kt-pair pieces so the first DoubleRow
    #     matmul can start as soon as its pair arrives; hi rides the sync
    #     queue, lo the scalar queue ---
    xh_sb = xw_pool.tile([128, 8, P], FP8, name="xh_sb")  # [part, ktile, pos]
    xl_sb = xw_pool.tile([128, 8, P], FP8, name="xl_sb")
    xh_r = x_hi[:, :].rearrange("(a p) c -> p a c", p=128)
    xl_r = x_lo[:, :].rearrange("(a p) c -> p a c", p=128)
    for tp in range(2):
        kp = slice(4 * tp, 4 * tp + 4)
        nc.sync.dma_start(out=xh_sb[:, kp, 0:512], in_=xh_r[:, kp, 0:512])
        nc.scalar.dma_start(out=xl_sb[:, kp, 0:512], in_=xl_r[:, kp, 0:512])
    nc.sync.dma_start(out=wkl_sb, in_=wk_lo[:, :].rearrange("(a p) c -> p a c", p=128))
    nc.scalar.dma_start(out=wql_sb, in_=wq_lo[:, :].rearrange("(a p) c -> p a c", p=128))
    nc.sync.dma_start(
        out=wvh_sb, in_=wv_hi[:, :].rearrange("(a p) c -> p a c", p=128)
    )
    nc.scalar.dma_start(
        out=wvl_sb, in_=wv_lo[:, :].rearrange("(a p) c -> p a c", p=128)
    )
    nc.gpsimd.dma_start(out=g.bo_sb, in_=g.bo[:, :])
    # late x-lo chunks ride sync: the scalar queue shares the Activation
    # sequencer, and dispatches there delay the first superiteration's exps
    for cb in range(1, 8):
        csl = slice(cb * 512, (cb + 1) * 512)
        nc.sync.dma_start(out=xh_sb[:, :, csl], in_=xh_r[:, :, csl])
        eng = nc.scalar if cb <= 2 else nc.sync
        eng.dma_start(out=xl_sb[:, :, csl], in_=xl_r[:, :, csl])

    # W_o isn't needed until the first output projection (~half-way in)
    wo_sb = ow_pool.tile([128, 8, D], BF16, name="wo_sb")
    woT_r = woT[:, :].rearrange("(a p) c -> p a c", p=128)
    nc.sync.dma_start(out=wo_sb[:, 0:4, :], in_=woT_r[:, 0:4, :])
    nc.scalar.dma_start(out=wo_sb[:, 4:8, :], in_=woT_r[:, 4:8, :])

    q_sb = qkv_pool.tile([128, P], BF16, name="q_sb")
    k_sb = qkv_pool.tile([128, P], BF16, name="k_sb")
    v_sb = qkv_pool.tile([128, 32, 130], BF16, name="v_sb")

    # Per-row-group A2A buffers: group m = rows m*1024 + c*128 .. +128.
    cc_in = [
        dram_pool.tile([NCORES * 128, 128], BF16, name=f"cc_in{m}") for m in range(4)
    ]
    tmp = [
        dram_pool.tile([NCORES * 128, 128], BF16, name=f"tmp{m}") for m in range(4)
    ]
    lw_all = [None] * 4

    # ---- filler units: one PSUM group each, emitted inside attention ----
    def dr3(ps, wh, wl, xh, xl, sl, last_stop):
        """12 DoubleRow matmuls: hi*hi + hi*lo + lo*hi over 4 kt-pairs."""
        terms = [(wh, xh), (wh, xl), (wl, xh)]
        for ti, (w, x) in enumerate(terms):
            for tp in range(4):
                kp = slice(2 * tp, 2 * tp + 2)
                nc.tensor.matmul(
                    out=ps,
                    lhsT=w[:, kp, :],
                    rhs=x[:, kp, sl],
                    start=(ti == 0 and tp == 0),
                    stop=(last_stop and ti == 2 and tp == 3),
                    perf_mode=DR,
                )

    def f_k(p8):
        def emit():
            sl = slice(p8 * 512, (p8 + 1) * 512)
            ps = ps_mm.tile([128, 512], F32, tag="mm", name="ps_k")
            dr3(ps, wkh_sb, wkl_sb, xh_sb, xl_sb, sl, last_stop=True)
            nc.vector.tensor_copy(out=k_sb[:, sl], in_=ps)
        return emit

    def f_q(p8):
        def emit():
            sl = slice(p8 * 512, (p8 + 1) * 512)
            ps = ps_mm.tile([128, 512], F32, tag="mm", name="ps_q")
            dr3(ps, wqh_sb, wql_sb, xh_sb, xl_sb, sl, last_stop=False)
            nc.tensor.matmul(
                out=ps, lhsT=bq_sb, rhs=ones512_sb, start=False, stop=True
            )
            nc.vector.tensor_copy(out=q_sb[:, sl], in_=ps)
        return emit

    def f_v(pt):
        def emit():
            psl = slice(pt * 128, (pt + 1) * 128)
            ps = ps_mm.tile([128, 130], F32, tag="mm", name="ps_v")
            terms = [(xh_sb, wvh_sb), (xh_sb, wvl_sb), (xl_sb, wvh_sb)]
            for ti, (x, w) in enumerate(terms):
                for tp in range(4):
                    kp = slice(2 * tp, 2 * tp + 2)
                    nc.tensor.matmul(
                        out=ps,
                        lhsT=x[:, kp, psl],
                        rhs=w[:, kp, :],
                        start=(ti == 0 and tp == 0),
                        stop=False,
                        perf_mode=DR,
                    )
            nc.tensor.matmul(
                out=ps, lhsT=ones_sb[:, 0:128], rhs=bv_sb, start=False, stop=True
            )
            with nc.allow_low_precision(reason="v copy with 1/64 descale"):
                nc.vector.tensor_scalar_mul(
                    out=v_sb[:, pt, :], in0=ps, scalar1=1.0 / WSCALE
                )
        return emit

    def f_lw(m, fast=False):
        def emit():
            t = obuf_pool.tile([128, 8, 128], BF16, tag=f"lw{m}", name="lw")
            src = tmp[m][:, :].rearrange("(a p) r -> p a r", p=128)
            if fast:
                # tail-critical load: quarters alternating between the two
                # HWDGE queues so the first kt-blocks land early
                for qt, eng in enumerate([nc.sync, nc.scalar, nc.sync,
                                          nc.scalar]):
                    eng.dma_start(out=t[:, 2 * qt : 2 * qt + 2, :],
                                  in_=src[:, 2 * qt : 2 * qt + 2, :])
            else:
                nc.gpsimd.dma_start(out=t[:, 0:4, :], in_=src[:, 0:4, :])
                nc.gpsimd.dma_start(out=t[:, 4:8, :], in_=src[:, 4:8, :])
            lw_all[m] = t
        return emit

    def f_op(m, nt, split_out=False):
        def emit():
            lw = lw_all[m]
            ps = ps_mm.tile([128, 512], F32, tag="mm", name="ps_o")
            for kt in range(8):
                nc.tensor.matmul(
                    out=ps, lhsT=lw[:, kt, :],
                    rhs=wo_sb[:, kt, nt * 512 : (nt + 1) * 512],
                    start=(kt == 0), stop=False,
                )
            nc.tensor.matmul(
                out=ps, lhsT=ones_sb, rhs=bo_sb[:, nt * 512 : (nt + 1) * 512],
                start=False, stop=True,
            )
            o_sb = obuf_pool.tile([128, 512], F32, tag="ob", name="o_sb")
            if split_out:
                for h, eng in enumerate([nc.sync, nc.scalar]):
                    sl = slice(h * 256, (h + 1) * 256)
                    nc.vector.tensor_copy(out=o_sb[:, sl], in_=ps[:, sl])
                    eng.dma_start(
                        out=out[m * 128 : (m + 1) * 128,
                                nt * 512 + h * 256 : nt * 512 + (h + 1) * 256],
                        in_=o_sb[:, sl],
                    )
            else:
                nc.vector.tensor_copy(out=o_sb, in_=ps)
                nc.sync.dma_start(
                    out=out[m * 128 : (m + 1) * 128,
                            nt * 512 : (nt + 1) * 512],
                    in_=o_sb,
                )
        return emit

    def emit_a2a(m):
        nc.gpsimd.collective_compute(
            "AllToAll",
            mybir.AluOpType.bypass,
            replica_groups=[list(range(NCORES))],
            ins=[cc_in[m][:].opt()],
            outs=[tmp[m][:].opt()],
        )

    # pending ctx matmul state: the ctx accumulation for key tile kt is
    # emitted one kt later (after the NEXT tile's logits) -- two tiles later
    # across a superiteration boundary -- so the PE never stalls on the
    # exp's completion semaphore or the previous si's PSUM drain.
    pend = []

    def flush_ctx():
        for b, kt, ps_c, et in pend:
            for hh in range(2):
                nc.tensor.matmul(
                    out=ps_c[hh],
                    lhsT=v_sb[:, b * 16 + kt, 65 * hh : 65 * hh + 65],
                    rhs=et[:, hh * 512 : (hh + 1) * 512],
                    start=(kt == 0),
                    stop=(kt == 15),
                )
        pend.clear()

    def emit_attn_part(b, qq, ps_c, kts, fillers=()):
        """Key tiles kts of one superiteration (both heads, q cols qq*512..)."""
        fillers = list(fillers)
        co = b * T
        qco = co + qq * 512
        nf = 0
        nkt = len(kts)
        for ki, kt in enumerate(kts):
            ps_l = ps_log.tile([128, 1024], F32, tag="log", name="ps_l")
            for hh in range(2):
                po = DK * hh
                nc.tensor.matmul(
                    out=ps_l[:, hh * 512 : (hh + 1) * 512],
                    lhsT=k_sb[po : po + DK, co + kt * 128 : co + (kt + 1) * 128],
                    rhs=q_sb[po : po + DK, qco : qco + 512],
                    start=True,
                    stop=True,
                )
            if kt != 1:  # lag-2 across the superiteration boundary
                flush_ctx()
            want = (ki + 1) * len(fillers) // nkt
            while nf < want:
                fillers[nf]()
                nf += 1
            et = et_pool.tile([128, 1024], BF16, tag="et", name="et")
            nc.scalar.activation(
                out=et, in_=ps_l,
                func=mybir.ActivationFunctionType.Exp,
                scale=EXP_SCALE,
            )
            pend.append((b, kt, ps_c, et))

    def emit_norm_copies(b, qq, ps_c):
        """DVE-only PSUM drain at the end of a superiteration (releases the
        ctx psum banks on the baseline schedule)."""
        flush_ctx()
        ctxr = norm_pool.tile([65, 1024], F32, tag="ctxr", name="ctxr")
        for hh in range(2):
            nc.vector.tensor_copy(
                out=ctxr[:, hh * 512 : (hh + 1) * 512], in_=ps_c[hh]
            )
        return ctxr

    def f_norm(b, qq, ctxr):
        """Deferred normalize, split in two fillers: [0] reciprocal (DVE
        only), [1] PE broadcast + mul + scatter.  Placing them a few key
        tiles apart in the next superiteration keeps the PE stream from
        ever waiting on the reciprocal."""
        m = 2 * b + qq // 2
        half = qq % 2
        rs = norm_pool.tile([65, 1024], BF16, tag="rsum", name="rs")

        def emit_recip():
            with nc.allow_low_precision(reason="softmax denom bf16 bcast"):
                nc.vector.reciprocal(out=rs[64:65, :], in_=ctxr[64:65, :])

        def emit_mul():
            ctxn = ctxn_pool.tile([64, 1024], BF16, tag="cn", name="ctxn")
            for hh in range(2):
                bc = ps_mm.tile([64, 512], F32, tag="mm", name="bc")
                nc.tensor.matmul(
                    out=bc,
                    lhsT=ones65_sb[64:65, 0:64],
                    rhs=rs[64:65, hh * 512 : (hh + 1) * 512],
                    start=True,
                    stop=True,
                )
                nc.vector.tensor_mul(
                    out=ctxn[:, hh * 512 : (hh + 1) * 512],
                    in0=ctxr[0:64, hh * 512 : (hh + 1) * 512],
                    in1=bc,
                )
                nc.sync.dma_start(
                    out=cc_in[m][:, :].rearrange("(j q) r -> q j r", q=128)[
                        DK * hh : DK * hh + DK, half * 4 : half * 4 + 4, :
                    ],
                    in_=ctxn[:, hh * 512 : (hh + 1) * 512].rearrange(
                        "f (j r) -> f j r", j=4
                    ),
                )
        return emit_recip, emit_mul

    def emit_norm_inline(b, qq, ps_c, prewarm=2):
        """Latency-critical norm (the very last superiteration): reciprocals
        read the colsum rows straight from PSUM, with PE keep-alive matmuls
        covering their latency."""
        flush_ctx()
        m = 2 * b + qq // 2
        half = qq % 2
        warm(prewarm)
        rs = norm_pool.tile([65, 1024], BF16, tag="rsum", name="rs")
        with nc.allow_low_precision(reason="softmax denom bf16 broadcast"):
            for hh in range(2):
                nc.vector.reciprocal(
                    out=rs[64:65, hh * 512 : (hh + 1) * 512],
                    in_=ps_c[hh][64:65, :],
                )
        ctxr = norm_pool.tile([65, 1024], F32, tag="ctxr", name="ctxr")
        for hh in range(2):
            nc.vector.tensor_copy(
                out=ctxr[:, hh * 512 : (hh + 1) * 512], in_=ps_c[hh]
            )
        ctxn = ctxn_pool.tile([64, 1024], BF16, tag="cn", name="ctxn")
        for hh in range(2):
            bc = ps_mm.tile([64, 512], F32, tag="mm", name="bc")
            nc.tensor.matmul(
                out=bc,
                lhsT=ones65_sb[64:65, 0:64],
                rhs=rs[64:65, hh * 512 : (hh + 1) * 512],
                start=True,
                stop=True,
            )
            nc.vector.tensor_mul(
                out=ctxn[:, hh * 512 : (hh + 1) * 512],
                in0=ctxr[0:64, hh * 512 : (hh + 1) * 512],
                in1=bc,
            )
            nc.sync.dma_start(
                out=cc_in[m][:, :].rearrange("(j q) r -> q j r", q=128)[
                    DK * hh : DK * hh + DK, half * 4 : half * 4 + 4, :
                ],
                in_=ctxn[:, hh * 512 : (hh + 1) * 512].rearrange(
                    "f (j r) -> f j r", j=4
                ),
            )

    def alloc_ps_c():
        return [
            ps_ctx.tile([65, 512], F32, tag="ctx", name=f"psc{hh}")
            for hh in range(2)
        ]

    def emit_attn(b, qq, fillers=()):
        """One full superiteration: both heads, q columns qq*512..+512.
        Returns three deferred fillers (flush+psum-drain, recip, broadcast+
        mul+scatter) for the next superiteration, so the si boundary never
        stalls the PE on the exp/normalize chain."""
        ps_c = alloc_ps_c()
        emit_attn_part(b, qq, ps_c, range(16), fillers)
        ctxr = norm_pool.tile([65, 1024], F32, tag="ctxr", name="ctxr")

        def copies():
            flush_ctx()
            for hh in range(2):
                nc.vector.tensor_copy(
                    out=ctxr[:, hh * 512 : (hh + 1) * 512], in_=ps_c[hh]
                )
        r, mu = f_norm(b, qq, ctxr)
        return copies, r, mu

    # fp32 warm source for PE keep-alive matmuls (throwaway work used to
    # cover reciprocal latencies and the final collective window -- a cold
    # PE restarts at half/quarter p-state)
    warm_src = obuf_pool.tile([128, 512], F32, tag="warm", name="warm_src")

    def warm(n):
        for _ in range(n):
            ps_d = ps_log.tile([128, 512], F32, tag="log", name="ps_warm")
            nc.tensor.matmul(
                out=ps_d, lhsT=warm_src[:, 0:128], rhs=warm_src,
                start=True, stop=True,
            )

        noop = lambda: None

    # ---- emission schedule ----
    # superiteration (0,0) is streamed in 4-kt blocks: each block's k slice
    # and v tiles are emitted (top level) just before the part that consumes
    # them, so attention starts as soon as x chunk 0 lands.  Output
    # projections trail their group's AllToAll by two superiterations (the
    # collective takes ~1.4 superiterations; anything earlier stalls the
    # in-order PE queue on the lw load).
    f_k(0)()
    f_q(0)()
    for pt in range(4):
        f_v(pt)()
    nc.vector.tensor_copy(out=warm_src, in_=q_sb[:, 0:512])
    ps_c00 = alloc_ps_c()
    for blk in range(4):
        if blk < 3:
            f_k(blk + 1)()
            for pt in range(4 * blk + 4, 4 * blk + 8):
                f_v(pt)()
        emit_attn_part(0, 0, ps_c00, range(4 * blk, 4 * blk + 4),
                       [f_q(1)] if blk == 3 else [])
    ctxr00 = emit_norm_copies(0, 0, ps_c00)
    r00, m00 = f_norm(0, 0, ctxr00)
    c01, r01, m01 = emit_attn(0, 1, [r00, m00, f_q(2), f_k(4), f_k(5),
                                     f_k(6), f_k(7)])
    c02, r02, m02 = emit_attn(0, 2, [c01, r01, m01, lambda: emit_a2a(0),
                                     f_q(3)]
                              + [f_v(pt) for pt in range(16, 24)])
    c03, r03, m03 = emit_attn(0, 3, [c02, r02, m02]
                              + [f_v(pt) for pt in range(24, 32)]
                              + [f_q(4), f_lw(0)])
    c10, r10, m10 = emit_attn(1, 0, [c03, r03, m03, f_q(5),
                                     f_op(0, 0), lambda: emit_a2a(1)])
    c11, r11, m11 = emit_attn(1, 1, [c10, r10, m10, f_q(6), f_op(0, 1),
                                     f_lw(1)])
    c12, r12, m12 = emit_attn(1, 2, [c11, r11, m11, f_q(7),
                                     f_op(1, 0), lambda: emit_a2a(2)])
    ps_c13 = alloc_ps_c()
    emit_attn_part(1, 3, ps_c13, range(16), [c12, r12, m12, f_op(1, 1), f_lw(2)])
    emit_norm_inline(1, 3, ps_c13, prewarm=2)
    emit_a2a(3)
    warm(3)  # bridges until lw2 lands (gated by the m2 collective)
    f_op(2, 0)()
    f_op(2, 1)()
    # keep the PE hot through the last collective's ~21us window
    warm(28)
    f_lw(3, fast=True)()
    f_op(3, 0)()
    f_op(3, 1, split_out=True)()


def _fp8_split(a):
    """a (f32) -> (hi, lo) fp8e4m3 with hi + lo ~= a."""
    hi = a.astype(NPFP8)
    lo = (a - hi.astype(np.float32)).astype(NPFP8)
    return hi, lo


def make_in_maps(x, W_qkv, b_qkv, W_o, b_o):
    x = np.asarray(x, dtype=np.float32)
    W_qkv = np.asarray(W_qkv, dtype=np.float32)
    b_qkv = np.asarray(b_qkv, dtype=np.float32)
    W_o = np.asarray(W_o, dtype=np.float32)
    b_o = np.asarray(b_o, dtype=np.float32)

    xT = np.ascontiguousarray(x.reshape(P, D).T)
    x_hi, x_lo = _fp8_split(xT)
    woT = np.ascontiguousarray(W_o.T).astype(NPBF16)
    # fold b_v into the output bias: out += W_o @ b_v
    bv_full = b_qkv[2 * D : 3 * D]
    bo_eff = (b_o + W_o @ bv_full).reshape(1, D).astype(NPBF16)

    in_maps = []
    for c in range(NCORES):
        wq = W_qkv[128 * c : 128 * c + 128]  # [128, 1024] q features
        wk = W_qkv[D + 128 * c : D + 128 * c + 128]
        wv = W_qkv[2 * D + 128 * c : 2 * D + 128 * c + 128]
        wvT_pad = np.zeros((D, 130), dtype=np.float32)
        wvT_pad[:, 0:64] = wv[0:64].T
        wvT_pad[:, 65:129] = wv[64:128].T
        # ones columns come from the bias matmul (value 64 -> 1.0 after the
        # 1/64 copy descale); v's real bias was folded into bo_eff.
        bv_pad = np.zeros((1, 130), dtype=np.float32)
        bv_pad[0, 64] = WSCALE
        bv_pad[0, 129] = WSCALE
        wq_hi, wq_lo = _fp8_split(WSCALE * np.ascontiguousarray(wq.T))
        wk_hi, wk_lo = _fp8_split(WSCALE * np.ascontiguousarray(wk.T))
        wv_hi, wv_lo = _fp8_split(WSCALE * wvT_pad)
        in_maps.append(
            {
                "x_hi": x_hi,
                "x_lo": x_lo,
                "wq_hi": wq_hi,
                "wq_lo": wq_lo,
                "wk_hi": wk_hi,
                "wk_lo": wk_lo,
                "wv_hi": wv_hi,
                "wv_lo": wv_lo,
                "bq": (WSCALE * b_qkv[128 * c : 128 * c + 128])
                .reshape(1, 128)
                .astype(NPBF16),
                "bv": bv_pad.astype(NPBF16),
                "woT": woT,
                "bo": bo_eff,
            }
        )
    return in_maps


def assemble_out(outs):
    """outs[c] is [512, 1024]: row tile rt holds global rows
    rt*1024 + c*128 .. +128 (interleaved ownership)."""
    full = np.zeros((P, D), dtype=np.float32)
    for c in range(NCORES):
        oc = np.asarray(outs[c], dtype=np.float32)
        for rt in range(4):
            full[rt * 1024 + c * 128 : rt * 1024 + c * 128 + 128] = oc[
                rt * 128 : (rt + 1) * 128
            ]
    return full.reshape(B, T, D)


_CACHED_GRAPH = None


def kernel(x, W_qkv, b_qkv, W_o, b_o):
    global _CACHED_GRAPH
    if _CACHED_GRAPH is None:
        _CACHED_GRAPH = build_graph()
    nc = _CACHED_GRAPH
    in_maps = make_in_maps(x, W_qkv, b_qkv, W_o, b_o)
    res = run_bass_kernel_spmd(nc, in_maps, core_ids=list(range(NCORES)))
    outs = [res.results[c]["out"] for c in range(NCORES)]
    return assemble_out(outs)


# revision 46
# speedup vs baseline: 1.0303x; 1.0261x over previous
"""Distributed multi-head attention kernel for one TRN2 chip (8 NeuronCores).

Problem: x[2,2048,1024] -> qkv proj (W_qkv[3072,1024], b_qkv) -> 16-head
attention (d_key=64) -> out proj (W_o[1024,1024], b_o).

Sharding: head tensor-parallel, 2 heads per core.  Everything on-device is
computed in transposed orientation so no transposes are ever needed:

  per core (heads h0=2c, h1=2c+1):
    qT/kT  [128, 4096]  (2 heads stacked on partitions; cols = b*2048+t),
        computed as  (64*W_q_local) @ x^T  in split-fp8 (see below); the
        64^2 logit scale is folded into the softmax exp's scale operand.
    v      [128pos x 32tile x 130]  natural orientation, with zero-weight /
        bias-64 "ones" columns at 64 and 129 (the 1/64 copy scale turns them
        into 1.0) so the ctx matmul's lhsT = [v_h | ones] produces the
        softmax denominator for free on psum partition 64.
    per (batch, q-quarter) "superiteration", both heads together:
        logitsT[keys,q] = kT-tile-as-lhsT @ qT  (K=64; head0 on PE rows
        0-63, head1 on rows 64-127)
        E = exp(logits/(8*64^2))   (no max subtraction: |l/8| < ~2.5)
        ctxT_unnorm[64,q] + colsum[1,q] accumulated over key tiles in PSUM;
        the ctx matmul for key tile kt is emitted one tile behind its exp
        (two behind across superiteration boundaries) so the in-order PE
        stream never waits on the Activation engine.
        normalize by PE outer-product broadcast of the reciprocal colsum;
        the psum drain / reciprocal / broadcast+mul run as three separate
        fillers inside the NEXT superiteration.

  Projection matmuls run as fp8e4m3 DoubleRow pairs (two 128-feature
  k-tiles per matmul at 0.5 cycles/row).  To stay within fp8's normal
  range the weights are pre-scaled by 64 on the host and split into
  hi + lo fp8 operands (x likewise split unscaled); three cross terms
  (hi*hi, hi*lo, lo*hi) recover ~bf16 accuracy at ~1/3 the PE cost.
  Exact-math bias folds: b_k is dropped entirely (softmax shift
  invariance), b_v is folded into b_o' = b_o + W_o @ b_v on the host,
  b_q is added via a [1,128]-lhsT bias matmul (scaled by 64).

  Output rows are owned interleaved: core c owns rows {m*1024 + c*128 + i,
  m=0..3}, so each row group m's AllToAll fires right after its two
  attention superiterations; the first three collectives and six of the
  eight output-projection groups hide under later attention (each f_op
  trails its collective by two superiterations -- the collective takes
  ~1.4 superiterations and an earlier in-order placement would stall the
  PE on the lw load).  Throwaway fp32 "warm" matmuls keep the PE's
  p-state at maximum through the final collective's ~21us window, and
  the queue assignment keeps data-gated DMAs off the scalar/vector
  queues (their sequencers are shared with the Activation/DVE engines).
  The host scatters the 8x[512,1024] outputs back to natural row order.

Matmul/compute dtype bf16/fp8-split (f32 PSUM accumulation); rel-err gate
is 2e-2.
"""

import sys

sys.path.insert(0, "/opt/trn_rl_repo")

import numpy as np
import ml_dtypes

import concourse.bass as bass
import concourse.tile as tile
from concourse import bacc, mybir
from concourse.bass_utils import run_bass_kernel_spmd

BF16 = mybir.dt.bfloat16
F32 = mybir.dt.float32
FP8 = mybir.dt.float8e4
NPBF16 = ml_dtypes.bfloat16
NPFP8 = ml_dtypes.float8_e4m3
DR = mybir.MatmulPerfMode.DoubleRow

D = 1024  # d_model
T = 2048  # seq len
B = 2  # batch
P = B * T  # 4096 total positions
H = 16  # total heads
DK = 64  # head dim
NCORES = 8
HL = H // NCORES  # 2 heads per core

WSCALE = 64.0  # host-side weight scale keeping fp8 operands in normal range
EXP_SCALE = 0.125 / (WSCALE * WSCALE)  # softmax exp scale incl. 64^2 logits


def build_graph(reps=1):
    nc = bacc.Bacc(
        "TRN2", target_bir_lowering=False, debug=False, num_devices=NCORES
    )

    # --- per-core external inputs ---
    x_hi = nc.declare_dram_parameter("x_hi", [D, P], FP8, isOutput=False)
    x_lo = nc.declare_dram_parameter("x_lo", [D, P], FP8, isOutput=False)
    wq_hi = nc.declare_dram_parameter("wq_hi", [D, 128], FP8, isOutput=False)
    wq_lo = nc.declare_dram_parameter("wq_lo", [D, 128], FP8, isOutput=False)
    wk_hi = nc.declare_dram_parameter("wk_hi", [D, 128], FP8, isOutput=False)
    wk_lo = nc.declare_dram_parameter("wk_lo", [D, 128], FP8, isOutput=False)
    wv_hi = nc.declare_dram_parameter("wv_hi", [D, 130], FP8, isOutput=False)
    wv_lo = nc.declare_dram_parameter("wv_lo", [D, 130], FP8, isOutput=False)
    bq = nc.declare_dram_parameter("bq", [128, 1], F32, isOutput=False)
    bv = nc.declare_dram_parameter("bv", [1, 130], BF16, isOutput=False)
    woT = nc.declare_dram_parameter("woT", [D, D], BF16, isOutput=False)
    bo = nc.declare_dram_parameter("bo", [1, D], BF16, isOutput=False)
    out = nc.declare_dram_parameter("out", [P // NCORES, D], F32, isOutput=True)

    with tile.TileContext(nc) as tc:
        with (
            tc.tile_pool(name="const", bufs=1) as const_pool,
            tc.tile_pool(name="xw", bufs=1) as xw_pool,
            tc.tile_pool(name="qkv", bufs=1) as qkv_pool,
            tc.tile_pool(name="et", bufs=3) as et_pool,
            tc.tile_pool(name="norm", bufs=2) as norm_pool,
            tc.tile_pool(name="ctxn", bufs=4) as ctxn_pool,
            tc.tile_pool(name="ow", bufs=2) as ow_pool,
            tc.tile_pool(name="obuf", bufs=2) as obuf_pool,
            tc.tile_pool(name="ps_mm", bufs=2, space="PSUM") as ps_mm,
            tc.tile_pool(name="ps_log", bufs=2, space="PSUM") as ps_log,
            tc.tile_pool(name="ps_ctx", bufs=2, space="PSUM") as ps_ctx,
            tc.tile_pool(name="dram", bufs=1, space="DRAM") as dram_pool,
        ):
            # tiny constants on the gpsimd queue so sync/scalar HWDGE start
            # on the weights/x immediately
            bq_sb = const_pool.tile([128, 1], F32)
            bv_sb = const_pool.tile([1, 130], BF16)
            bo_sb = const_pool.tile([1, D], BF16)
            nc.gpsimd.dma_start(out=bv_sb, in_=bv[:, :])
            nc.gpsimd.dma_start(out=bq_sb, in_=bq[:, :])
            ones_sb = const_pool.tile([1, 128], BF16)
            nc.vector.memset(ones_sb, 1.0)
            ones512_sb = const_pool.tile([1, 512], BF16)
            nc.vector.memset(ones512_sb, 1.0)
            ones65_sb = const_pool.tile([65, 128], BF16)
            nc.vector.memset(ones65_sb, 1.0)

            env = dict(locals())
            env.pop("env", None)
            for rep in range(reps):
                emit_body(nc, tc, env)

    nc.compile()
    return nc


def emit_body(nc, tc, env):
    """Emit one full forward pass.

    Engine instruction streams are in-order, so the emission schedule is a
    hand software-pipeline.  Attention processes BOTH local heads together
    per 512-column q-quarter; QKV projection groups and output-projection
    groups are interleaved as fillers into the attention kt-loops.
    """
    g = type("G", (), env)  # attribute access to captured bindings

    x_hi, x_lo = g.x_hi, g.x_lo
    wq_hi, wq_lo, wk_hi, wk_lo = g.wq_hi, g.wq_lo, g.wk_hi, g.wk_lo
    wv_hi, wv_lo, woT, out = g.wv_hi, g.wv_lo, g.woT, g.out
    bq_sb, bv_sb, bo_sb = g.bq_sb, g.bv_sb, g.bo_sb
    ones_sb, ones512_sb, ones65_sb = g.ones_sb, g.ones512_sb, g.ones65_sb
    xw_pool, qkv_pool = g.xw_pool, g.qkv_pool
    et_pool, norm_pool, ctxn_pool = g.et_pool, g.norm_pool, g.ctxn_pool
    ow_pool, obuf_pool = g.ow_pool, g.obuf_pool
    ps_mm, ps_log, ps_ctx, dram_pool = g.ps_mm, g.ps_log, g.ps_ctx, g.dram_pool

    # --- weights into SBUF first (k/q weights gate the first projections) ---
    wkh_sb = xw_pool.tile([128, 8, 128], FP8, name="wkh_sb")
    wkl_sb = xw_pool.tile([128, 8, 128], FP8, name="wkl_sb")
    wqh_sb = xw_pool.tile([128, 8, 128], FP8, name="wqh_sb")
    wql_sb = xw_pool.tile([128, 8, 128], FP8, name="wql_sb")
    wvh_sb = xw_pool.tile([128, 8, 130], FP8, name="wvh_sb")
    wvl_sb = xw_pool.tile([128, 8, 130], FP8, name="wvl_sb")
    nc.sync.dma_start(out=wkh_sb, in_=wk_hi[:, :].rearrange("(a p) c -> p a c", p=128))

    # --- x^T hi/lo; chunk 0 lands as kt-pair pieces so the first DoubleRow
    #     matmul can start as soon as its pair arrives; hi rides the sync
    #     queue, lo the scalar queue ---
    xh_sb = xw_pool.tile([128, 8, P], FP8, name="xh_sb")  # [part, ktile, pos]
    xl_sb = xw_pool.tile([128, 8, P], FP8, name="xl_sb")
    xh_r = x_hi[:, :].rearrange("(a p) c -> p a c", p=128)
    xl_r = x_lo[:, :].rearrange("(a p) c -> p a c", p=128)
    nc.scalar.dma_start(out=xl_sb[:, 0:4, 0:512], in_=xl_r[:, 0:4, 0:512])
    nc.sync.dma_start(out=xh_sb[:, 0:4, 0:512], in_=xh_r[:, 0:4, 0:512])
    nc.scalar.dma_start(out=wqh_sb, in_=wq_hi[:, :].rearrange("(a p) c -> p a c", p=128))
    nc.sync.dma_start(out=xh_sb[:, 4:8, 0:512], in_=xh_r[:, 4:8, 0:512])
    nc.scalar.dma_start(out=xl_sb[:, 4:8, 0:512], in_=xl_r[:, 4:8, 0:512])
    nc.sync.dma_start(out=wkl_sb, in_=wk_lo[:, :].rearrange("(a p) c -> p a c", p=128))
    nc.scalar.dma_start(out=wql_sb, in_=wq_lo[:, :].rearrange("(a p) c -> p a c", p=128))
    nc.sync.dma_start(
        out=wvh_sb, in_=wv_hi[:, :].rearrange("(a p) c -> p a c", p=128)
    )
    nc.scalar.dma_start(
        out=wvl_sb, in_=wv_lo[:, :].rearrange("(a p) c -> p a c", p=128)
    )
    nc.gpsimd.dma_start(out=g.bo_sb, in_=g.bo[:, :])
    # late x-lo chunks ride sync: the scalar queue shares the Activation
    # sequencer, and dispatches there delay the first superiteration's exps
    for cb in range(1, 8):
        csl = slice(cb * 512, (cb + 1) * 512)
        nc.sync.dma_start(out=xh_sb[:, :, csl], in_=xh_r[:, :, csl])
        eng = nc.scalar if cb <= 2 else nc.sync
        eng.dma_start(out=xl_sb[:, :, csl], in_=xl_r[:, :, csl])

    # W_o isn't needed until the first output projection (~half-way in)
    wo_sb = ow_pool.tile([128, 8, D], BF16, name="wo_sb")
    woT_r = woT[:, :].rearrange("(a p) c -> p a c", p=128)
    nc.sync.dma_start(out=wo_sb[:, 0:4, :], in_=woT_r[:, 0:4, :])
    nc.scalar.dma_start(out=wo_sb[:, 4:8, :], in_=woT_r[:, 4:8, :])

    q_sb = qkv_pool.tile([128, P], BF16, name="q_sb")
    k_sb = qkv_pool.tile([128, P], BF16, name="k_sb")
    v_sb = qkv_pool.tile([128, 32, 130], BF16, name="v_sb")

    # Per-row-group A2A buffers: group m = rows m*1024 + c*128 .. +128.
    cc_in = [
        dram_pool.tile([NCORES * 128, 128], BF16, name=f"cc_in{m}") for m in range(4)
    ]
    tmp = [
        dram_pool.tile([NCORES * 128, 128], BF16, name=f"tmp{m}") for m in range(4)
    ]
    lw_all = [None] * 4

    # ---- filler units: one PSUM group each, emitted inside attention ----
    def dr3(ps, wh, wl, xh, xl, sl, last_stop):
        """12 DoubleRow matmuls: hi*hi + hi*lo + lo*hi over 4 kt-pairs."""
        terms = [(wh, xh), (wh, xl), (wl, xh)]
        for ti, (w, x) in enumerate(terms):
            for tp in range(4):
                kp = slice(2 * tp, 2 * tp + 2)
                nc.tensor.matmul(
                    out=ps,
                    lhsT=w[:, kp, :],
                    rhs=x[:, kp, sl],
                    start=(ti == 0 and tp == 0),
                    stop=(last_stop and ti == 2 and tp == 3),
                    perf_mode=DR,
                )

    def f_k(p8):
        def emit():
            sl = slice(p8 * 512, (p8 + 1) * 512)
            ps = ps_mm.tile([128, 512], F32, tag="mm", name="ps_k")
            dr3(ps, wkh_sb, wkl_sb, xh_sb, xl_sb, sl, last_stop=True)
            nc.vector.tensor_copy(out=k_sb[:, sl], in_=ps)
        return emit

    def f_q(p8):
        def emit():
            sl = slice(p8 * 512, (p8 + 1) * 512)
            ps = ps_mm.tile([128, 512], F32, tag="mm", name="ps_q")
            dr3(ps, wqh_sb, wql_sb, xh_sb, xl_sb, sl, last_stop=True)
            with nc.allow_low_precision(reason="q bias add in bf16 drain"):
                nc.vector.tensor_scalar(
                    out=q_sb[:, sl], in0=ps, scalar1=bq_sb, scalar2=None,
                    op0=mybir.AluOpType.add,
                )
        return emit

    def f_v(pt):
        def emit():
            psl = slice(pt * 128, (pt + 1) * 128)
            ps = ps_mm.tile([128, 130], F32, tag="mm", name="ps_v")
            terms = [(xh_sb, wvh_sb), (xh_sb, wvl_sb), (xl_sb, wvh_sb)]
            for ti, (x, w) in enumerate(terms):
                for tp in range(4):
                    kp = slice(2 * tp, 2 * tp + 2)
                    nc.tensor.matmul(
                        out=ps,
                        lhsT=x[:, kp, psl],
                        rhs=w[:, kp, :],
                        start=(ti == 0 and tp == 0),
                        stop=False,
                        perf_mode=DR,
                    )
            nc.tensor.matmul(
                out=ps, lhsT=ones_sb[:, 0:128], rhs=bv_sb, start=False, stop=True
            )
            with nc.allow_low_precision(reason="v copy with 1/64 descale"):
                nc.vector.tensor_scalar_mul(
                    out=v_sb[:, pt, :], in0=ps, scalar1=1.0 / WSCALE
                )
        return emit

    def f_lw(m, fast=False):
        def emit():
            t = obuf_pool.tile([128, 8, 128], BF16, tag=f"lw{m}", name="lw")
            src = tmp[m][:, :].rearrange("(a p) r -> p a r", p=128)
            if fast:
                # tail-critical load: quarters alternating between the two
                # HWDGE queues so the first kt-blocks land early
                for qt, eng in enumerate([nc.sync, nc.scalar, nc.sync,
                                          nc.scalar]):
                    eng.dma_start(out=t[:, 2 * qt : 2 * qt + 2, :],
                                  in_=src[:, 2 * qt : 2 * qt + 2, :])
            else:
                nc.gpsimd.dma_start(out=t[:, 0:4, :], in_=src[:, 0:4, :])
                nc.gpsimd.dma_start(out=t[:, 4:8, :], in_=src[:, 4:8, :])
            lw_all[m] = t
        return emit

    def f_op(m, nt, split_out=False):
        def emit():
            lw = lw_all[m]
            ps = ps_mm.tile([128, 512], F32, tag="mm", name="ps_o")
            for kt in range(8):
                nc.tensor.matmul(
                    out=ps, lhsT=lw[:, kt, :],
                    rhs=wo_sb[:, kt, nt * 512 : (nt + 1) * 512],
                    start=(kt == 0), stop=(kt == 7),
                )
            o_sb = obuf_pool.tile([128, 512], F32, tag="ob", name="o_sb")
            nsl = slice(nt * 512, (nt + 1) * 512)
            if split_out:
                for h, eng in enumerate([nc.sync, nc.scalar]):
                    sl = slice(h * 256, (h + 1) * 256)
                    nc.vector.scalar_tensor_tensor(
                        out=o_sb[:, sl], in0=ps[:, sl], scalar=1.0,
                        in1=bo_full[:, nt * 512 + h * 256 : nt * 512 + (h + 1) * 256],
                        op0=mybir.AluOpType.mult, op1=mybir.AluOpType.add,
                    )
                    eng.dma_start(
                        out=out[m * 128 : (m + 1) * 128,
                                nt * 512 + h * 256 : nt * 512 + (h + 1) * 256],
                        in_=o_sb[:, sl],
                    )
            else:
                nc.vector.scalar_tensor_tensor(
                    out=o_sb, in0=ps, scalar=1.0, in1=bo_full[:, nsl],
                    op0=mybir.AluOpType.mult, op1=mybir.AluOpType.add,
                )
                nc.sync.dma_start(
                    out=out[m * 128 : (m + 1) * 128, nsl], in_=o_sb
                )
        return emit

    def emit_a2a(m):
        nc.gpsimd.collective_compute(
            "AllToAll",
            mybir.AluOpType.bypass,
            replica_groups=[list(range(NCORES))],
            ins=[cc_in[m][:].opt()],
            outs=[tmp[m][:].opt()],
        )

    # pending ctx matmul state: the ctx accumulation for key tile kt is
    # emitted one kt later (after the NEXT tile's logits) -- two tiles later
    # across a superiteration boundary -- so the PE never stalls on the
    # exp's completion semaphore or the previous si's PSUM drain.
    pend = []

    def flush_ctx():
        for b, kt, ps_c, et in pend:
            for hh in range(2):
                nc.tensor.matmul(
                    out=ps_c[hh],
                    lhsT=v_sb[:, b * 16 + kt, 65 * hh : 65 * hh + 65],
                    rhs=et[:, hh * 512 : (hh + 1) * 512],
                    start=(kt == 0),
                    stop=(kt == 15),
                )
        pend.clear()

    def emit_attn_part(b, qq, ps_c, kts, fillers=()):
        """Key tiles kts of one superiteration (both heads, q cols qq*512..)."""
        fillers = list(fillers)
        co = b * T
        qco = co + qq * 512
        nf = 0
        nkt = len(kts)
        for ki, kt in enumerate(kts):
            ps_l = ps_log.tile([128, 1024], F32, tag="log", name="ps_l")
            for hh in range(2):
                po = DK * hh
                nc.tensor.matmul(
                    out=ps_l[:, hh * 512 : (hh + 1) * 512],
                    lhsT=k_sb[po : po + DK, co + kt * 128 : co + (kt + 1) * 128],
                    rhs=q_sb[po : po + DK, qco : qco + 512],
                    start=True,
                    stop=True,
                )
            if kt != 1:  # lag-2 across the superiteration boundary
                flush_ctx()
            want = (ki + 1) * len(fillers) // nkt
            while nf < want:
                fillers[nf]()
                nf += 1
            et = et_pool.tile([128, 1024], BF16, tag="et", name="et")
            nc.scalar.activation(
                out=et, in_=ps_l,
                func=mybir.ActivationFunctionType.Exp,
                scale=EXP_SCALE,
            )
            pend.append((b, kt, ps_c, et))

    def emit_norm_copies(b, qq, ps_c):
        """DVE-only PSUM drain at the end of a superiteration (releases the
        ctx psum banks on the baseline schedule)."""
        flush_ctx()
        ctxr = norm_pool.tile([65, 1024], F32, tag="ctxr", name="ctxr")
        for hh in range(2):
            nc.vector.tensor_copy(
                out=ctxr[:, hh * 512 : (hh + 1) * 512], in_=ps_c[hh]
            )
        return ctxr

    def f_norm(b, qq, ctxr):
        """Deferred normalize, split in two fillers: [0] reciprocal (DVE
        only), [1] PE broadcast + mul + scatter.  Placing them a few key
        tiles apart in the next superiteration keeps the PE stream from
        ever waiting on the reciprocal."""
        m = 2 * b + qq // 2
        half = qq % 2
        rs = norm_pool.tile([65, 1024], BF16, tag="rsum", name="rs")

        def emit_recip():
            with nc.allow_low_precision(reason="softmax denom bf16 bcast"):
                nc.vector.reciprocal(out=rs[64:65, :], in_=ctxr[64:65, :])

        def emit_mul():
            ctxn = ctxn_pool.tile([64, 1024], BF16, tag="cn", name="ctxn")
            for hh in range(2):
                bc = ps_mm.tile([64, 512], F32, tag="mm", name="bc")
                nc.tensor.matmul(
                    out=bc,
                    lhsT=ones65_sb[64:65, 0:64],
                    rhs=rs[64:65, hh * 512 : (hh + 1) * 512],
                    start=True,
                    stop=True,
                )
                nc.vector.tensor_mul(
                    out=ctxn[:, hh * 512 : (hh + 1) * 512],
                    in0=ctxr[0:64, hh * 512 : (hh + 1) * 512],
                    in1=bc,
                )
                nc.sync.dma_start(
                    out=cc_in[m][:, :].rearrange("(j q) r -> q j r", q=128)[
                        DK * hh : DK * hh + DK, half * 4 : half * 4 + 4, :
                    ],
                    in_=ctxn[:, hh * 512 : (hh + 1) * 512].rearrange(
                        "f (j r) -> f j r", j=4
                    ),
                )
        return emit_recip, emit_mul

    def emit_norm_inline(b, qq, ps_c, prewarm=2):
        """Latency-critical norm (the very last superiteration): reciprocals
        read the colsum rows straight from PSUM, with PE keep-alive matmuls
        covering their latency."""
        flush_ctx()
        m = 2 * b + qq // 2
        half = qq % 2
        warm(prewarm)
        rs = norm_pool.tile([65, 1024], BF16, tag="rsum", name="rs")
        with nc.allow_low_precision(reason="softmax denom bf16 broadcast"):
            for hh in range(2):
                nc.vector.reciprocal(
                    out=rs[64:65, hh * 512 : (hh + 1) * 512],
                    in_=ps_c[hh][64:65, :],
                )
        ctxr = norm_pool.tile([65, 1024], F32, tag="ctxr", name="ctxr")
        for hh in range(2):
            nc.vector.tensor_copy(
                out=ctxr[:, hh * 512 : (hh + 1) * 512], in_=ps_c[hh]
            )
        ctxn = ctxn_pool.tile([64, 1024], BF16, tag="cn", name="ctxn")
        for hh in range(2):
            bc = ps_mm.tile([64, 512], F32, tag="mm", name="bc")
            nc.tensor.matmul(
                out=bc,
                lhsT=ones65_sb[64:65, 0:64],
                rhs=rs[64:65, hh * 512 : (hh + 1) * 512],
                start=True,
                stop=True,
            )
            nc.vector.tensor_mul(
                out=ctxn[:, hh * 512 : (hh + 1) * 512],
                in0=ctxr[0:64, hh * 512 : (hh + 1) * 512],
                in1=bc,
            )
            nc.sync.dma_start(
                out=cc_in[m][:, :].rearrange("(j q) r -> q j r", q=128)[
                    DK * hh : DK * hh + DK, half * 4 : half * 4 + 4, :
                ],
                in_=ctxn[:, hh * 512 : (hh + 1) * 512].rearrange(
                    "f (j r) -> f j r", j=4
                ),
            )

    def alloc_ps_c():
        return [
            ps_ctx.tile([65, 512], F32, tag="ctx", name=f"psc{hh}")
            for hh in range(2)
        ]

    def emit_attn(b, qq, fillers=()):
        """One full superiteration: both heads, q columns qq*512..+512.
        Returns three deferred fillers (flush+psum-drain, recip, broadcast+
        mul+scatter) for the next superiteration, so the si boundary never
        stalls the PE on the exp/normalize chain."""
        ps_c = alloc_ps_c()
        emit_attn_part(b, qq, ps_c, range(16), fillers)
        ctxr = norm_pool.tile([65, 1024], F32, tag="ctxr", name="ctxr")

        def copies():
            flush_ctx()
            for hh in range(2):
                nc.vector.tensor_copy(
                    out=ctxr[:, hh * 512 : (hh + 1) * 512], in_=ps_c[hh]
                )
        r, mu = f_norm(b, qq, ctxr)
        return copies, r, mu

    # fp32 warm source for PE keep-alive matmuls (throwaway work used to
    # cover reciprocal latencies and the final collective window -- a cold
    # PE restarts at half/quarter p-state)
    warm_src = obuf_pool.tile([128, 512], F32, tag="warm", name="warm_src")
    bo_full = ow_pool.tile([128, D], BF16, name="bo_full")

    def emit_bo_full():
        for half in range(2):
            ps = ps_mm.tile([128, 512], F32, tag="mm", name="ps_bo")
            nc.tensor.matmul(
                out=ps, lhsT=ones_sb,
                rhs=bo_sb[:, half * 512 : (half + 1) * 512],
                start=True, stop=True,
            )
            with nc.allow_low_precision(reason="bias broadcast to bf16"):
                nc.vector.tensor_copy(
                    out=bo_full[:, half * 512 : (half + 1) * 512], in_=ps
                )

    def warm(n):
        for _ in range(n):
            ps_d = ps_log.tile([128, 512], F32, tag="log", name="ps_warm")
            nc.tensor.matmul(
                out=ps_d, lhsT=warm_src[:, 0:128], rhs=warm_src,
                start=True, stop=True,
            )

        noop = lambda: None

    # ---- emission schedule ----
    # superiteration (0,0) is streamed in 4-kt blocks: each block's k slice
    # and v tiles are emitted (top level) just before the part that consumes
    # them, so attention starts as soon as x chunk 0 lands.  Output
    # projections trail their group's AllToAll by two superiterations (the
    # collective takes ~1.4 superiterations; anything earlier stalls the
    # in-order PE queue on the lw load).
    f_k(0)()
    f_q(0)()
    for pt in range(4):
        f_v(pt)()
    nc.vector.tensor_copy(out=warm_src, in_=q_sb[:, 0:512])
    emit_bo_full()
    ps_c00 = alloc_ps_c()
    for blk in range(4):
        if blk < 3:
            f_k(blk + 1)()
            for pt in range(4 * blk + 4, 4 * blk + 8):
                f_v(pt)()
        emit_attn_part(0, 0, ps_c00, range(4 * blk, 4 * blk + 4),
                       [f_q(1)] if blk == 3 else [])
    ctxr00 = emit_norm_copies(0, 0, ps_c00)
    r00, m00 = f_norm(0, 0, ctxr00)
    c01, r01, m01 = emit_attn(0, 1, [r00, m00, f_q(2), f_k(4), f_k(5),
                                     f_k(6), f_k(7)])
    c02, r02, m02 = emit_attn(0, 2, [c01, r01, m01, lambda: emit_a2a(0),
                                     f_q(3)]
                              + [f_v(pt) for pt in range(16, 24)])
    c03, r03, m03 = emit_attn(0, 3, [c02, r02, m02]
                              + [f_v(pt) for pt in range(24, 32)]
                              + [f_q(4), f_lw(0)])
    c10, r10, m10 = emit_attn(1, 0, [c03, r03, m03, f_q(5),
                                     lambda: emit_a2a(1)])
    c11, r11, m11 = emit_attn(1, 1, [c10, r10, m10, f_q(6), f_lw(1)])
    c12, r12, m12 = emit_attn(1, 2, [c11, r11, m11, f_q(7),
                                     lambda: emit_a2a(2)])
    ps_c13 = alloc_ps_c()
    emit_attn_part(1, 3, ps_c13, range(16), [c12, r12, m12, f_lw(2)])
    emit_norm_inline(1, 3, ps_c13, prewarm=2)
    emit_a2a(3)
    # the last collective's ~21us window hosts ALL trailing output
    # projections (their lw loads completed long before); the leftover is
    # bridged with warm matmuls so the final projection runs at full p-state
    warm(3)  # bridges until lw2 lands (gated by the m2 collective)
    f_op(2, 0)()
    f_op(2, 1)()
    f_op(0, 0)()
    f_op(0, 1)()
    f_op(1, 0)()
    f_op(1, 1)()
    warm(21)
    f_lw(3, fast=True)()
    f_op(3, 0)()
    f_op(3, 1, split_out=True)()


def _fp8_split(a):
    """a (f32) -> (hi, lo) fp8e4m3 with hi + lo ~= a."""
    hi = a.astype(NPFP8)
    lo = (a - hi.astype(np.float32)).astype(NPFP8)
    return hi, lo


def make_in_maps(x, W_qkv, b_qkv, W_o, b_o):
    x = np.asarray(x, dtype=np.float32)
    W_qkv = np.asarray(W_qkv, dtype=np.float32)
    b_qkv = np.asarray(b_qkv, dtype=np.float32)
    W_o = np.asarray(W_o, dtype=np.float32)
    b_o = np.asarray(b_o, dtype=np.float32)

    xT = np.ascontiguousarray(x.reshape(P, D).T)
    x_hi, x_lo = _fp8_split(xT)
    woT = np.ascontiguousarray(W_o.T).astype(NPBF16)
    # fold b_v into the output bias: out += W_o @ b_v
    bv_full = b_qkv[2 * D : 3 * D]
    bo_eff = (b_o + W_o @ bv_full).reshape(1, D).astype(NPBF16)

    in_maps = []
    for c in range(NCORES):
        wq = W_qkv[128 * c : 128 * c + 128]  # [128, 1024] q features
        wk = W_qkv[D + 128 * c : D + 128 * c + 128]
        wv = W_qkv[2 * D + 128 * c : 2 * D + 128 * c + 128]
        wvT_pad = np.zeros((D, 130), dtype=np.float32)
        wvT_pad[:, 0:64] = wv[0:64].T
        wvT_pad[:, 65:129] = wv[64:128].T
        # ones columns come from the bias matmul (value 64 -> 1.0 after the
        # 1/64 copy descale); v's real bias was folded into bo_eff.
        bv_pad = np.zeros((1, 130), dtype=np.float32)
        bv_pad[0, 64] = WSCALE
        bv_pad[0, 129] = WSCALE
        wq_hi, wq_lo = _fp8_split(WSCALE * np.ascontiguousarray(wq.T))
        wk_hi, wk_lo = _fp8_split(WSCALE * np.ascontiguousarray(wk.T))
        wv_hi, wv_lo = _fp8_split(WSCALE * wvT_pad)
        in_maps.append(
            {
                "x_hi": x_hi,
                "x_lo": x_lo,
                "wq_hi": wq_hi,
                "wq_lo": wq_lo,
                "wk_hi": wk_hi,
                "wk_lo": wk_lo,
                "wv_hi": wv_hi,
                "wv_lo": wv_lo,
                "bq": (WSCALE * b_qkv[128 * c : 128 * c + 128])
                .reshape(128, 1)
                .astype(np.float32),
                "bv": bv_pad.astype(NPBF16),
                "woT": woT,
                "bo": bo_eff,
            }
        )
    return in_maps


def assemble_out(outs):
    """outs[c] is [512, 1024]: row tile rt holds global rows
    rt*1024 + c*128 .. +128 (interleaved ownership)."""
    full = np.zeros((P, D), dtype=np.float32)
    for c in range(NCORES):
        oc = np.asarray(outs[c], dtype=np.float32)
        for rt in range(4):
            full[rt * 1024 + c * 128 : rt * 1024 + c * 128 + 128] = oc[
                rt * 128 : (rt + 1) * 128
            ]
    return full.reshape(B, T, D)


_CACHED_GRAPH = None


def kernel(x, W_qkv, b_qkv, W_o, b_o):
    global _CACHED_GRAPH
    if _CACHED_GRAPH is None:
        _CACHED_GRAPH = build_graph()
    nc = _CACHED_GRAPH
    in_maps = make_in_maps(x, W_qkv, b_qkv, W_o, b_o)
    res = run_bass_kernel_spmd(nc, in_maps, core_ids=list(range(NCORES)))
    outs = [res.results[c]["out"] for c in range(NCORES)]
    return assemble_out(outs)
